# revision 8
# baseline (speedup 1.0000x reference)
"""Trainium2 kernel for BottomUpAttention (gnn_message_passing).

Math note: the reference applies softmax over a singleton axis
(``softmax(scores[:, None], axis=1)``), which is identically 1.0 for every
cell, so the attention branch (cell_keys / tissue_q / tanh / attn_w) cannot
affect the output.  The module reduces exactly to

    out = tissue_features + segment_sum(cell_features, cluster_assignments)

which is a memory-bound scatter-add over the cell features.

Strategy (8 NeuronCores, SPMD, no collectives):
  * Shard by *tissue*: each core owns 625 tissues, grouped into 20 blocks of
    32.  Tissues are greedily packed into blocks by descending cell count so
    every block has a near-equal number of cells (minimises padding).
  * Host argsorts cells by block and packs each block's cells into 128-row
    tiles, padded to a common tile count T_b so all cores run the identical
    SPMD program.
  * Cell rows are quantized on the host to fp8 e3m4 (4 mantissa bits).  The
    resulting segment sums land at ~1.4e-2 max relative error — inside the
    2e-2 tolerance — while streaming only 1 byte/element from HBM.
  * On device, each 128-cell tile is reduced by a one-hot matmul into its
    block's [32, 256] fp32 PSUM accumulator: lhsT[i, j] = (localid[i] == j).
    Blocks are processed three at a time, one per 128x32 column tile of the
    PE array (tile_position is inferred from the PSUM quarter), so the
    matmul streams overlap and the PE is not serialized on the 256-column
    moving operand.
  * One-hots for a whole block are built by a single DVE
    tensor_tensor(is_equal) comparing a constant iota row against a 0-step
    broadcast of the local ids.  PSUM quarters are drained by the scalar
    engine (keeping the DVE free), giving [128, 5*256] per core; the host
    inverse-permutes rows into the final [5000, 256] and adds
    tissue_features there.
"""

import numpy as np

P = 128          # SBUF partitions / matmul contraction dim
NCORES = 8
BLK = 32         # tissues per block (PSUM partition rows per column tile)
NPAR = 3         # blocks in flight (one per 128x32 PE column tile;
                 # PSUM AP base partition only encodes 0/32/64, so the
                 # fourth quarter is unusable)

DATA_DT = "f8"   # "f8" (fp8 e3m4 cell data) or "f16" (fp16 cell data)

LAST_RESULTS = None  # BassKernelResults of the most recent kernel() call

_PROGRAM_CACHE = {}


def _build_program(NT, T_b, NBLK, DIM):
    import concourse.mybir as mybir
    import concourse.tile as tile
    from concourse import bacc

    f32 = mybir.dt.float32
    f16 = mybir.dt.float16
    xdt = mybir.dt.float8e3 if DATA_DT == "f8" else f16
    NGRP = NBLK // NPAR

    nc = bacc.Bacc(
        "TRN2",
        target_bir_lowering=False,
        debug=False,
        enable_asserts=False,
        num_devices=NCORES,
    )
    # cell data, partition-major: x[p, t, 4j:4j+4] = features of cell
    # (t*128 + p), bitcast to f32 words so the DMA moves 4-byte elements
    # (1-byte-element descriptors run at a reduced rate)
    xw = DIM * mybir.dt.size(xdt) // 4
    x = nc.dram_tensor("x", [P, NT, xw], f32, kind="ExternalInput")
    loc = nc.dram_tensor("loc", [P, NT], f16, kind="ExternalInput")
    iota = nc.dram_tensor("iota", [P, T_b * BLK], f16, kind="ExternalInput")
    NROW = NPAR * BLK
    y = nc.dram_tensor("y", [NROW, NGRP * DIM], f32, kind="ExternalOutput")

    with tile.TileContext(nc) as tc:
        with (
            tc.tile_pool(name="const", bufs=1) as cpool,
            tc.tile_pool(name="data", bufs=3) as dpool,
            tc.tile_pool(name="oh", bufs=6) as ohpool,
            tc.tile_pool(name="psum", bufs=2, space="PSUM") as ppool,
        ):
            iota_sb = cpool.tile([P, T_b * BLK], f16)
            nc.scalar.dma_start(out=iota_sb[:], in_=iota[:])
            loc_sb = cpool.tile([P, NT], f16)
            nc.scalar.dma_start(out=loc_sb[:], in_=loc[:])
            out_sb = cpool.tile([NROW, NGRP * DIM], f32)

            for g in range(NGRP):
                dt_ = dpool.tile([P, NPAR * T_b, xw], f32, tag="data")
                nc.sync.dma_start(
                    out=dt_[:],
                    in_=x[:, g * NPAR * T_b : (g + 1) * NPAR * T_b, :],
                )
                ohs = []
                for h in range(NPAR):
                    b = g * NPAR + h
                    oh = ohpool.tile([P, T_b, BLK], xdt, tag="oh")
                    nc.vector.tensor_tensor(
                        out=oh[:],
                        in0=iota_sb[:].rearrange("p (k c) -> p k c", k=T_b),
                        in1=loc_sb[:, b * T_b : (b + 1) * T_b]
                        .rearrange("p (k o) -> p k o", o=1)
                        .to_broadcast([P, T_b, BLK]),
                        op=mybir.AluOpType.is_equal,
                    )
                    ohs.append(oh)
                ps = ppool.tile([P, DIM], f32, tag="ps")
                for t in range(T_b):
                    for h in range(NPAR):
                        nc.tensor.matmul(
                            out=ps[h * BLK : (h + 1) * BLK, :],
                            lhsT=ohs[h][:, t, :],
                            rhs=dt_[:, h * T_b + t, :].bitcast(xdt),
                            start=(t == 0),
                            stop=(t == T_b - 1),
                        )
                osl = out_sb[:, g * DIM : (g + 1) * DIM]
                nc.scalar.copy(out=osl, in_=ps[:NROW, :])
                nc.scalar.dma_start(
                    out=y[:, g * DIM : (g + 1) * DIM], in_=osl
                )
    nc.compile()
    return nc


def kernel(
    cell_features,
    tissue_features,
    cluster_assignments,
    W_cell,
    b_cell,
    W_tissue,
    b_tissue,
    attn_w,
):
    global LAST_RESULTS
    import ml_dtypes
    from concourse.bass_utils import run_bass_kernel_spmd

    cells = np.asarray(cell_features, dtype=np.float32)
    tissue = np.asarray(tissue_features, dtype=np.float32)
    assign = np.asarray(cluster_assignments).astype(np.int64)

    n_cell, DIM = cells.shape
    n_tissue = tissue.shape[0]
    TPC = -(-n_tissue // NCORES)          # tissues per core (ceil)
    NBLK = -(-TPC // BLK)                 # blocks per core
    NBLK = -(-NBLK // NPAR) * NPAR        # round to a multiple of NPAR
    nblocks_g = NCORES * NBLK

    np_xdt = ml_dtypes.float8_e3m4 if DATA_DT == "f8" else np.float16
    xq = cells.astype(np_xdt)

    # ---- host: balance tissues into blocks by cell count (less padding) ----
    tcounts = np.bincount(assign, minlength=n_tissue)
    t_order_desc = np.argsort(-tcounts, kind="stable")
    block_sum = np.zeros(nblocks_g, dtype=np.int64)
    block_fill = np.zeros(nblocks_g, dtype=np.int64)
    tissue2block = np.empty(n_tissue, dtype=np.int64)
    tissue2loc = np.empty(n_tissue, dtype=np.int64)
    import heapq

    heap = [(0, b) for b in range(nblocks_g)]
    heapq.heapify(heap)
    for t in t_order_desc:
        while True:
            s, b = heapq.heappop(heap)
            if block_fill[b] < BLK:
                break
        tissue2block[t] = b
        tissue2loc[t] = block_fill[b]
        block_fill[b] += 1
        block_sum[b] += tcounts[t]
        if block_fill[b] < BLK:
            heapq.heappush(heap, (block_sum[b], b))

    T_b = max(1, int(-(-block_sum.max() // P)))  # tiles per block (all cores)
    CAP = T_b * P
    NT = NBLK * T_b

    # ---- host: sort cells by (block, position) and pack per core ----
    cell_block = tissue2block[assign]
    order = np.argsort(cell_block, kind="stable").astype(np.int64)
    sorted_block = cell_block[order]
    cuts = np.searchsorted(sorted_block, np.arange(nblocks_g + 1))
    loc_of_cell = tissue2loc[assign].astype(np.float16)

    iota_np = np.ascontiguousarray(
        np.tile(np.arange(BLK, dtype=np.float16), (P, T_b))
    )

    in_maps = []
    for k in range(NCORES):
        pi = np.zeros(NBLK * CAP, dtype=np.int64)
        lo_ids = np.full(NBLK * CAP, float(BLK), dtype=np.float16)  # pad -> no hit
        for b in range(NBLK):
            i = k * NBLK + b
            seg = order[cuts[i] : cuts[i + 1]]
            pi[b * CAP : b * CAP + len(seg)] = seg
            lo_ids[b * CAP : b * CAP + len(seg)] = loc_of_cell[seg]
        # partition-major: x[p, t, :] = xq[pi[t*P + p]], viewed as f32 words
        x = np.ascontiguousarray(xq[pi.reshape(NT, P).T]).view(np.float32)
        locT = np.ascontiguousarray(lo_ids.reshape(NT, P).T)
        in_maps.append({"x": x, "loc": locT, "iota": iota_np})

    # ---- device program (cached on tiling geometry) ----
    key = (NT, T_b, NBLK, DIM, DATA_DT)
    nc = _PROGRAM_CACHE.get(key)
    if nc is None:
        nc = _build_program(NT, T_b, NBLK, DIM)
        _PROGRAM_CACHE[key] = nc

    res = run_bass_kernel_spmd(nc, in_maps, core_ids=list(range(NCORES)))
    LAST_RESULTS = res

    # ---- host: inverse-permute per-core outputs into [n_tissue, DIM] ----
    NGRP = NBLK // NPAR
    yb = np.concatenate(
        [
            res.results[k]["y"]
            .reshape(NPAR, BLK, NGRP, DIM)
            .transpose(2, 0, 1, 3)
            .reshape(NBLK, BLK, DIM)
            for k in range(NCORES)
        ],
        axis=0,
    )  # [nblocks_g, BLK, DIM] in (block, localid) layout
    out = np.ascontiguousarray(yb[tissue2block, tissue2loc]) + tissue
    return out


# revision 9
# speedup vs baseline: 1.2015x; 1.2015x over previous
"""Trainium2 kernel for BottomUpAttention (gnn_message_passing).

Math note: the reference applies softmax over a singleton axis
(``softmax(scores[:, None], axis=1)``), which is identically 1.0 for every
cell, so the attention branch (cell_keys / tissue_q / tanh / attn_w) cannot
affect the output.  The module reduces exactly to

    out = tissue_features + segment_sum(cell_features, cluster_assignments)

which is a memory-bound scatter-add over the cell features.

Strategy (8 NeuronCores, SPMD, no collectives):
  * Shard by *tissue*: each core owns 625 tissues, grouped into 20 blocks of
    32.  Tissues are greedily packed into blocks by descending cell count so
    every block has a near-equal number of cells (minimises padding).
  * Host argsorts cells by block and packs each block's cells into 128-row
    tiles, padded to a common tile count T_b so all cores run the identical
    SPMD program.
  * Cell rows are quantized on the host to fp8 e3m4 (4 mantissa bits).  The
    resulting segment sums land at ~1.4e-2 max relative error — inside the
    2e-2 tolerance — while streaming only 1 byte/element from HBM.
  * On device, each 128-cell tile is reduced by a one-hot matmul into its
    block's [32, 256] fp32 PSUM accumulator: lhsT[i, j] = (localid[i] == j).
    Blocks are processed three at a time, one per 128x32 column tile of the
    PE array (tile_position is inferred from the PSUM quarter), so the
    matmul streams overlap and the PE is not serialized on the 256-column
    moving operand.
  * One-hots for a whole block are built by a single DVE
    tensor_tensor(is_equal) comparing a constant iota row against a 0-step
    broadcast of the local ids.  PSUM quarters are drained by the scalar
    engine (keeping the DVE free), giving [128, 5*256] per core; the host
    inverse-permutes rows into the final [5000, 256] and adds
    tissue_features there.
"""

import numpy as np

P = 128          # SBUF partitions / matmul contraction dim
NCORES = 8
BLK = 32         # tissues per block (PSUM partition rows per column tile)
NPAR = 3         # blocks in flight (one per 128x32 PE column tile;
                 # PSUM AP base partition only encodes 0/32/64, so the
                 # fourth quarter is unusable)

DATA_DT = "f8"   # "f8" (fp8 e3m4 cell data) or "f16" (fp16 cell data)

LAST_RESULTS = None  # BassKernelResults of the most recent kernel() call

_PROGRAM_CACHE = {}


def _build_program(NT, T_b, NBLK, DIM):
    import concourse.mybir as mybir
    import concourse.tile as tile
    from concourse import bacc

    f32 = mybir.dt.float32
    f16 = mybir.dt.float16
    xdt = mybir.dt.float8e3 if DATA_DT == "f8" else f16
    NGRP = NBLK // NPAR

    nc = bacc.Bacc(
        "TRN2",
        target_bir_lowering=False,
        debug=False,
        enable_asserts=False,
        num_devices=NCORES,
    )
    # cell data, partition-major: x[p, t, 4j:4j+4] = features of cell
    # (t*128 + p), bitcast to f32 words so the DMA moves 4-byte elements
    # (1-byte-element descriptors run at a reduced rate)
    xw = DIM * mybir.dt.size(xdt) // 4
    x = nc.dram_tensor("x", [P, NT, xw], f32, kind="ExternalInput")
    loc = nc.dram_tensor("loc", [P, NT], f16, kind="ExternalInput")
    iota = nc.dram_tensor("iota", [P, T_b * BLK], f16, kind="ExternalInput")
    NROW = NPAR * BLK
    y = nc.dram_tensor("y", [NROW, NGRP * DIM], f32, kind="ExternalOutput")

    with tile.TileContext(nc) as tc:
        with (
            tc.tile_pool(name="const", bufs=1) as cpool,
            tc.tile_pool(name="data", bufs=9) as dpool,
            tc.tile_pool(name="oh", bufs=6) as ohpool,
            tc.tile_pool(name="psum", bufs=2, space="PSUM") as ppool,
        ):
            iota_sb = cpool.tile([P, T_b * BLK], f16)
            nc.scalar.dma_start(out=iota_sb[:], in_=iota[:])
            loc_sb = cpool.tile([P, NT], f16)
            nc.scalar.dma_start(out=loc_sb[:], in_=loc[:])
            out_sb = cpool.tile([NROW, NGRP * DIM], f32)

            for g in range(NGRP):
                dts = []
                ohs = []
                for h in range(NPAR):
                    b = g * NPAR + h
                    dt_ = dpool.tile([P, T_b, xw], f32, tag="data")
                    nc.sync.dma_start(
                        out=dt_[:], in_=x[:, b * T_b : (b + 1) * T_b, :]
                    )
                    dts.append(dt_)
                for h in range(NPAR):
                    b = g * NPAR + h
                    oh = ohpool.tile([P, T_b, BLK], xdt, tag="oh")
                    nc.vector.tensor_tensor(
                        out=oh[:],
                        in0=iota_sb[:].rearrange("p (k c) -> p k c", k=T_b),
                        in1=loc_sb[:, b * T_b : (b + 1) * T_b]
                        .rearrange("p (k o) -> p k o", o=1)
                        .to_broadcast([P, T_b, BLK]),
                        op=mybir.AluOpType.is_equal,
                    )
                    ohs.append(oh)
                ps = ppool.tile([P, DIM], f32, tag="ps")
                for t in range(T_b):
                    for h in range(NPAR):
                        nc.tensor.matmul(
                            out=ps[h * BLK : (h + 1) * BLK, :],
                            lhsT=ohs[h][:, t, :],
                            rhs=dts[h][:, t, :].bitcast(xdt),
                            start=(t == 0),
                            stop=(t == T_b - 1),
                        )
                osl = out_sb[:, g * DIM : (g + 1) * DIM]
                nc.scalar.copy(out=osl, in_=ps[:NROW, :])
                nc.scalar.dma_start(
                    out=y[:, g * DIM : (g + 1) * DIM], in_=osl
                )
    nc.compile()
    return nc


def kernel(
    cell_features,
    tissue_features,
    cluster_assignments,
    W_cell,
    b_cell,
    W_tissue,
    b_tissue,
    attn_w,
):
    global LAST_RESULTS
    import ml_dtypes
    from concourse.bass_utils import run_bass_kernel_spmd

    cells = np.asarray(cell_features, dtype=np.float32)
    tissue = np.asarray(tissue_features, dtype=np.float32)
    assign = np.asarray(cluster_assignments).astype(np.int64)

    n_cell, DIM = cells.shape
    n_tissue = tissue.shape[0]
    TPC = -(-n_tissue // NCORES)          # tissues per core (ceil)
    NBLK = -(-TPC // BLK)                 # blocks per core
    NBLK = -(-NBLK // NPAR) * NPAR        # round to a multiple of NPAR
    nblocks_g = NCORES * NBLK

    np_xdt = ml_dtypes.float8_e3m4 if DATA_DT == "f8" else np.float16
    xq = cells.astype(np_xdt)

    # ---- host: balance tissues into blocks by cell count (less padding) ----
    tcounts = np.bincount(assign, minlength=n_tissue)
    t_order_desc = np.argsort(-tcounts, kind="stable")
    block_sum = np.zeros(nblocks_g, dtype=np.int64)
    block_fill = np.zeros(nblocks_g, dtype=np.int64)
    tissue2block = np.empty(n_tissue, dtype=np.int64)
    tissue2loc = np.empty(n_tissue, dtype=np.int64)
    import heapq

    heap = [(0, b) for b in range(nblocks_g)]
    heapq.heapify(heap)
    for t in t_order_desc:
        while True:
            s, b = heapq.heappop(heap)
            if block_fill[b] < BLK:
                break
        tissue2block[t] = b
        tissue2loc[t] = block_fill[b]
        block_fill[b] += 1
        block_sum[b] += tcounts[t]
        if block_fill[b] < BLK:
            heapq.heappush(heap, (block_sum[b], b))

    T_b = max(1, int(-(-block_sum.max() // P)))  # tiles per block (all cores)
    CAP = T_b * P
    NT = NBLK * T_b

    # ---- host: sort cells by (block, position) and pack per core ----
    cell_block = tissue2block[assign]
    order = np.argsort(cell_block, kind="stable").astype(np.int64)
    sorted_block = cell_block[order]
    cuts = np.searchsorted(sorted_block, np.arange(nblocks_g + 1))
    loc_of_cell = tissue2loc[assign].astype(np.float16)

    iota_np = np.ascontiguousarray(
        np.tile(np.arange(BLK, dtype=np.float16), (P, T_b))
    )

    in_maps = []
    for k in range(NCORES):
        pi = np.zeros(NBLK * CAP, dtype=np.int64)
        lo_ids = np.full(NBLK * CAP, float(BLK), dtype=np.float16)  # pad -> no hit
        for b in range(NBLK):
            i = k * NBLK + b
            seg = order[cuts[i] : cuts[i + 1]]
            pi[b * CAP : b * CAP + len(seg)] = seg
            lo_ids[b * CAP : b * CAP + len(seg)] = loc_of_cell[seg]
        # partition-major: x[p, t, :] = xq[pi[t*P + p]], viewed as f32 words
        x = np.ascontiguousarray(xq[pi.reshape(NT, P).T]).view(np.float32)
        locT = np.ascontiguousarray(lo_ids.reshape(NT, P).T)
        in_maps.append({"x": x, "loc": locT, "iota": iota_np})

    # ---- device program (cached on tiling geometry) ----
    key = (NT, T_b, NBLK, DIM, DATA_DT)
    nc = _PROGRAM_CACHE.get(key)
    if nc is None:
        nc = _build_program(NT, T_b, NBLK, DIM)
        _PROGRAM_CACHE[key] = nc

    res = run_bass_kernel_spmd(nc, in_maps, core_ids=list(range(NCORES)))
    LAST_RESULTS = res

    # ---- host: inverse-permute per-core outputs into [n_tissue, DIM] ----
    NGRP = NBLK // NPAR
    yb = np.concatenate(
        [
            res.results[k]["y"]
            .reshape(NPAR, BLK, NGRP, DIM)
            .transpose(2, 0, 1, 3)
            .reshape(NBLK, BLK, DIM)
            for k in range(NCORES)
        ],
        axis=0,
    )  # [nblocks_g, BLK, DIM] in (block, localid) layout
    out = np.ascontiguousarray(yb[tissue2block, tissue2loc]) + tissue
    return out


# revision 11
# speedup vs baseline: 1.2162x; 1.0123x over previous
"""Trainium2 kernel for BottomUpAttention (gnn_message_passing).

Math note: the reference applies softmax over a singleton axis
(``softmax(scores[:, None], axis=1)``), which is identically 1.0 for every
cell, so the attention branch (cell_keys / tissue_q / tanh / attn_w) cannot
affect the output.  The module reduces exactly to

    out = tissue_features + segment_sum(cell_features, cluster_assignments)

which is a memory-bound scatter-add over the cell features.

Strategy (8 NeuronCores, SPMD, no collectives):
  * Shard by *tissue*: each core owns 625 tissues, grouped into 20 blocks of
    32.  Tissues are greedily packed into blocks by descending cell count so
    every block has a near-equal number of cells (minimises padding).
  * Host argsorts cells by block and packs each block's cells into 128-row
    tiles, padded to a common tile count T_b so all cores run the identical
    SPMD program.
  * Cell rows are quantized on the host to fp8 e3m4 (4 mantissa bits).  The
    resulting segment sums land at ~1.4e-2 max relative error — inside the
    2e-2 tolerance — while streaming only 1 byte/element from HBM.
  * On device, each 128-cell tile is reduced by a one-hot matmul into its
    block's [32, 256] fp32 PSUM accumulator: lhsT[i, j] = (localid[i] == j).
    Blocks are processed three at a time, one per 128x32 column tile of the
    PE array (tile_position is inferred from the PSUM quarter), so the
    matmul streams overlap and the PE is not serialized on the 256-column
    moving operand.
  * One-hots for a whole block are built by a single DVE
    tensor_tensor(is_equal) comparing a constant iota row against a 0-step
    broadcast of the local ids.  PSUM quarters are drained by the scalar
    engine (keeping the DVE free), giving [128, 5*256] per core; the host
    inverse-permutes rows into the final [5000, 256] and adds
    tissue_features there.
"""

import numpy as np

P = 128          # SBUF partitions / matmul contraction dim
NCORES = 8
BLK = 32         # tissues per block (PSUM partition rows per column tile)
NPAR = 3         # blocks in flight (one per 128x32 PE column tile;
                 # PSUM AP base partition only encodes 0/32/64, so the
                 # fourth quarter is unusable)

DATA_DT = "f8"   # "f8" (fp8 e3m4 cell data) or "f16" (fp16 cell data)

LAST_RESULTS = None  # BassKernelResults of the most recent kernel() call

_PROGRAM_CACHE = {}


def _build_program(NT, T_b, NBLK, DIM):
    import concourse.mybir as mybir
    import concourse.tile as tile
    from concourse import bacc

    f32 = mybir.dt.float32
    f16 = mybir.dt.float16
    xdt = mybir.dt.float8e3 if DATA_DT == "f8" else f16
    NGRP = NBLK // NPAR

    nc = bacc.Bacc(
        "TRN2",
        target_bir_lowering=False,
        debug=False,
        enable_asserts=False,
        num_devices=NCORES,
    )
    # cell data, partition-major: x[p, t, 4j:4j+4] = features of cell
    # (t*128 + p), bitcast to f32 words so the DMA moves 4-byte elements
    # (1-byte-element descriptors run at a reduced rate)
    xw = DIM * mybir.dt.size(xdt) // 4
    x = nc.dram_tensor("x", [P, NT, xw], f32, kind="ExternalInput")
    loc = nc.dram_tensor("loc", [P, NT // 2], f32, kind="ExternalInput")
    iota = nc.dram_tensor("iota", [P, T_b * BLK // 2], f32, kind="ExternalInput")
    NROW = NPAR * BLK
    y = nc.dram_tensor("y", [NROW, NGRP * DIM], f32, kind="ExternalOutput")

    with tile.TileContext(nc) as tc:
        with (
            tc.tile_pool(name="const", bufs=1) as cpool,
            tc.tile_pool(name="data", bufs=9) as dpool,
            tc.tile_pool(name="oh", bufs=6) as ohpool,
            tc.tile_pool(name="psum", bufs=2, space="PSUM") as ppool,
        ):
            iota_sb = cpool.tile([P, T_b * BLK // 2], f32)
            nc.scalar.dma_start(out=iota_sb[:], in_=iota[:])
            loc_sb = cpool.tile([P, NT // 2], f32)
            nc.scalar.dma_start(out=loc_sb[:], in_=loc[:])
            iota_f16 = iota_sb[:].bitcast(f16)
            loc_f16 = loc_sb[:].bitcast(f16)
            out_sb = cpool.tile([NROW, NGRP * DIM], f32)

            for g in range(NGRP):
                dts = []
                ohs = []
                # Final group: load each block in two halves so its matmuls
                # start before the last bytes land (shorter drain tail).
                nsplit = 2 if (g == NGRP - 1 and T_b >= 4) else 1
                Th = T_b // 2 if nsplit == 2 else T_b
                for s in range(nsplit):
                    lo, hi = (0, Th) if s == 0 else (Th, T_b)
                    for h in range(NPAR):
                        b = g * NPAR + h
                        dt_ = dpool.tile([P, hi - lo, xw], f32, tag="data")
                        nc.sync.dma_start(
                            out=dt_[:],
                            in_=x[:, b * T_b + lo : b * T_b + hi, :],
                        )
                        dts.append(dt_)
                for h in range(NPAR):
                    b = g * NPAR + h
                    oh = ohpool.tile([P, T_b, BLK], xdt, tag="oh")
                    nc.vector.tensor_tensor(
                        out=oh[:],
                        in0=iota_f16.rearrange("p (k c) -> p k c", k=T_b),
                        in1=loc_f16[:, b * T_b : (b + 1) * T_b]
                        .rearrange("p (k o) -> p k o", o=1)
                        .to_broadcast([P, T_b, BLK]),
                        op=mybir.AluOpType.is_equal,
                    )
                    ohs.append(oh)
                ps = ppool.tile([P, DIM], f32, tag="ps")
                for t in range(T_b):
                    for h in range(NPAR):
                        if t < Th:
                            rhs = dts[h][:, t, :]
                        else:
                            rhs = dts[NPAR + h][:, t - Th, :]
                        nc.tensor.matmul(
                            out=ps[h * BLK : (h + 1) * BLK, :],
                            lhsT=ohs[h][:, t, :],
                            rhs=rhs.bitcast(xdt),
                            start=(t == 0),
                            stop=(t == T_b - 1),
                        )
                osl = out_sb[:, g * DIM : (g + 1) * DIM]
                nc.scalar.copy(out=osl, in_=ps[:NROW, :])
                nc.scalar.dma_start(
                    out=y[:, g * DIM : (g + 1) * DIM], in_=osl
                )
    nc.compile()
    return nc


def kernel(
    cell_features,
    tissue_features,
    cluster_assignments,
    W_cell,
    b_cell,
    W_tissue,
    b_tissue,
    attn_w,
):
    global LAST_RESULTS
    import ml_dtypes
    from concourse.bass_utils import run_bass_kernel_spmd

    cells = np.asarray(cell_features, dtype=np.float32)
    tissue = np.asarray(tissue_features, dtype=np.float32)
    assign = np.asarray(cluster_assignments).astype(np.int64)

    n_cell, DIM = cells.shape
    n_tissue = tissue.shape[0]
    TPC = -(-n_tissue // NCORES)          # tissues per core (ceil)
    NBLK = -(-TPC // BLK)                 # blocks per core
    NBLK = -(-NBLK // NPAR) * NPAR        # round to a multiple of NPAR
    nblocks_g = NCORES * NBLK

    np_xdt = ml_dtypes.float8_e3m4 if DATA_DT == "f8" else np.float16
    xq = cells.astype(np_xdt)

    # ---- host: balance tissues into blocks by cell count (less padding) ----
    tcounts = np.bincount(assign, minlength=n_tissue)
    t_order_desc = np.argsort(-tcounts, kind="stable")
    block_sum = np.zeros(nblocks_g, dtype=np.int64)
    block_fill = np.zeros(nblocks_g, dtype=np.int64)
    tissue2block = np.empty(n_tissue, dtype=np.int64)
    tissue2loc = np.empty(n_tissue, dtype=np.int64)
    import heapq

    heap = [(0, b) for b in range(nblocks_g)]
    heapq.heapify(heap)
    for t in t_order_desc:
        while True:
            s, b = heapq.heappop(heap)
            if block_fill[b] < BLK:
                break
        tissue2block[t] = b
        tissue2loc[t] = block_fill[b]
        block_fill[b] += 1
        block_sum[b] += tcounts[t]
        if block_fill[b] < BLK:
            heapq.heappush(heap, (block_sum[b], b))

    T_b = max(2, int(-(-block_sum.max() // P)))  # tiles per block (all cores)
    T_b += T_b % 2  # keep NT even so loc packs into f32 words
    CAP = T_b * P
    NT = NBLK * T_b

    # ---- host: sort cells by (block, position) and pack per core ----
    cell_block = tissue2block[assign]
    order = np.argsort(cell_block, kind="stable").astype(np.int64)
    sorted_block = cell_block[order]
    cuts = np.searchsorted(sorted_block, np.arange(nblocks_g + 1))
    loc_of_cell = tissue2loc[assign].astype(np.float16)

    iota_f32 = np.ascontiguousarray(
        np.tile(np.arange(BLK, dtype=np.float16), (P, T_b))
    ).view(np.float32)

    in_maps = []
    for k in range(NCORES):
        pi = np.zeros(NBLK * CAP, dtype=np.int64)
        lo_ids = np.full(NBLK * CAP, float(BLK), dtype=np.float16)  # pad -> no hit
        for b in range(NBLK):
            i = k * NBLK + b
            seg = order[cuts[i] : cuts[i + 1]]
            pi[b * CAP : b * CAP + len(seg)] = seg
            lo_ids[b * CAP : b * CAP + len(seg)] = loc_of_cell[seg]
        # partition-major: x[p, t, :] = xq[pi[t*P + p]], viewed as f32 words
        x = np.ascontiguousarray(xq[pi.reshape(NT, P).T]).view(np.float32)
        locT = np.ascontiguousarray(lo_ids.reshape(NT, P).T).view(np.float32)
        in_maps.append({"x": x, "loc": locT, "iota": iota_f32})

    # ---- device program (cached on tiling geometry) ----
    key = (NT, T_b, NBLK, DIM, DATA_DT)
    nc = _PROGRAM_CACHE.get(key)
    if nc is None:
        nc = _build_program(NT, T_b, NBLK, DIM)
        _PROGRAM_CACHE[key] = nc

    res = run_bass_kernel_spmd(nc, in_maps, core_ids=list(range(NCORES)))
    LAST_RESULTS = res

    # ---- host: inverse-permute per-core outputs into [n_tissue, DIM] ----
    NGRP = NBLK // NPAR
    yb = np.concatenate(
        [
            res.results[k]["y"]
            .reshape(NPAR, BLK, NGRP, DIM)
            .transpose(2, 0, 1, 3)
            .reshape(NBLK, BLK, DIM)
            for k in range(NCORES)
        ],
        axis=0,
    )  # [nblocks_g, BLK, DIM] in (block, localid) layout
    out = np.ascontiguousarray(yb[tissue2block, tissue2loc]) + tissue
    return out


# revision 12
# speedup vs baseline: 1.2293x; 1.0108x over previous
"""Trainium2 kernel for BottomUpAttention (gnn_message_passing).

Math note: the reference applies softmax over a singleton axis
(``softmax(scores[:, None], axis=1)``), which is identically 1.0 for every
cell, so the attention branch (cell_keys / tissue_q / tanh / attn_w) cannot
affect the output.  The module reduces exactly to

    out = tissue_features + segment_sum(cell_features, cluster_assignments)

which is a memory-bound scatter-add over the cell features.

Strategy (8 NeuronCores, SPMD, no collectives):
  * Shard by *tissue*: each core owns 625 tissues, grouped into 21 blocks of
    32.  Tissues are greedily packed into blocks by descending cell count so
    every block nearly fills its capacity (minimises padding).  Blocks are
    processed in groups of three; per-group tile counts T_bs[g] are chosen
    so total capacity only slightly exceeds the cell count, and the last
    group is the smallest (shortest drain tail).
  * Host argsorts cells by block and packs each block's cells into 128-row
    tiles; all cores run the identical SPMD program.
  * Cell rows are quantized on the host to fp8 e3m4 (4 mantissa bits).  The
    resulting segment sums land at ~1.4e-2 max relative error — inside the
    2e-2 tolerance — while streaming only 1 byte/element from HBM.  The
    DMA reads them as f32 words (1-byte-element descriptors run ~10%
    slower); matmuls bitcast back to fp8.
  * On device, each 128-cell tile is reduced by a one-hot matmul into its
    block's [32, 256] fp32 PSUM accumulator: lhsT[i, j] = (localid[i] == j).
    The three blocks of a group map to three 128x32 column tiles of the PE
    array (tile_position is inferred from the PSUM quarter; the fourth
    quarter is unreachable — PSUM AP base partition only encodes 0/32/64),
    so the matmul streams overlap and the PE is not serialized on the
    256-column moving operand.
  * One-hots for a whole block are built by a single DVE
    tensor_tensor(is_equal) comparing a constant iota row against a 0-step
    broadcast of the local ids.  PSUM is drained by the scalar engine
    (keeping the DVE free) and each group's [96, 256] slab is written back
    to DRAM immediately; the host inverse-permutes rows into the final
    [5000, 256] and adds tissue_features there.
  * The final group's data is fetched in two half-transfers per block so
    its matmuls start before the last bytes land.
"""

import numpy as np

P = 128          # SBUF partitions / matmul contraction dim
NCORES = 8
BLK = 32         # tissues per block (PSUM partition rows per column tile)
NPAR = 3         # blocks in flight (one per 128x32 PE column tile;
                 # PSUM AP base partition only encodes 0/32/64, so the
                 # fourth quarter is unusable)

DATA_DT = "f8"   # "f8" (fp8 e3m4 cell data) or "f16" (fp16 cell data)

LAST_RESULTS = None  # BassKernelResults of the most recent kernel() call

_PROGRAM_CACHE = {}


def _build_program(T_bs, NBLK, DIM):
    import concourse.mybir as mybir
    import concourse.tile as tile
    from concourse import bacc

    f32 = mybir.dt.float32
    f16 = mybir.dt.float16
    xdt = mybir.dt.float8e3 if DATA_DT == "f8" else f16
    NGRP = NBLK // NPAR
    NT = NPAR * sum(T_bs)
    NTL = NT + (NT & 1)          # loc length, padded even for f32 packing
    Tmax = max(T_bs)

    nc = bacc.Bacc(
        "TRN2",
        target_bir_lowering=False,
        debug=False,
        enable_asserts=False,
        num_devices=NCORES,
    )
    # cell data, partition-major: x[p, t, 4j:4j+4] = features of cell
    # (t*128 + p), bitcast to f32 words so the DMA moves 4-byte elements
    # (1-byte-element descriptors run at a reduced rate)
    xw = DIM * mybir.dt.size(xdt) // 4
    x = nc.dram_tensor("x", [P, NT, xw], f32, kind="ExternalInput")
    loc = nc.dram_tensor("loc", [P, NTL // 2], f32, kind="ExternalInput")
    iota = nc.dram_tensor("iota", [P, Tmax * BLK // 2], f32, kind="ExternalInput")
    NROW = NPAR * BLK
    y = nc.dram_tensor("y", [NROW, NGRP * DIM], f32, kind="ExternalOutput")

    with tile.TileContext(nc) as tc:
        with (
            tc.tile_pool(name="const", bufs=1) as cpool,
            tc.tile_pool(name="data", bufs=9) as dpool,
            tc.tile_pool(name="oh", bufs=6) as ohpool,
            tc.tile_pool(name="psum", bufs=2, space="PSUM") as ppool,
        ):
            iota_sb = cpool.tile([P, Tmax * BLK // 2], f32)
            nc.scalar.dma_start(out=iota_sb[:], in_=iota[:])
            loc_sb = cpool.tile([P, NTL // 2], f32)
            nc.scalar.dma_start(out=loc_sb[:], in_=loc[:])
            iota_f16 = iota_sb[:].bitcast(f16)
            loc_f16 = loc_sb[:].bitcast(f16)
            out_sb = cpool.tile([NROW, NGRP * DIM], f32)

            off = 0  # tile offset of the current group's first block
            for g in range(NGRP):
                T_b = T_bs[g]
                dts = []
                ohs = []
                # Final group: load each block in two halves so its matmuls
                # start before the last bytes land (shorter drain tail).
                nsplit = 2 if (g == NGRP - 1 and T_b >= 4) else 1
                Th = T_b // 2 if nsplit == 2 else T_b
                for s in range(nsplit):
                    lo, hi = (0, Th) if s == 0 else (Th, T_b)
                    for h in range(NPAR):
                        t0 = off + h * T_b
                        dt_ = dpool.tile([P, hi - lo, xw], f32, tag="data")
                        nc.sync.dma_start(
                            out=dt_[:], in_=x[:, t0 + lo : t0 + hi, :]
                        )
                        dts.append(dt_)
                for h in range(NPAR):
                    t0 = off + h * T_b
                    oh = ohpool.tile([P, T_b, BLK], xdt, tag="oh")
                    nc.vector.tensor_tensor(
                        out=oh[:],
                        in0=iota_f16[:, : T_b * BLK].rearrange(
                            "p (k c) -> p k c", k=T_b
                        ),
                        in1=loc_f16[:, t0 : t0 + T_b]
                        .rearrange("p (k o) -> p k o", o=1)
                        .to_broadcast([P, T_b, BLK]),
                        op=mybir.AluOpType.is_equal,
                    )
                    ohs.append(oh)
                ps = ppool.tile([P, DIM], f32, tag="ps")
                for t in range(T_b):
                    for h in range(NPAR):
                        if t < Th:
                            rhs = dts[h][:, t, :]
                        else:
                            rhs = dts[NPAR + h][:, t - Th, :]
                        nc.tensor.matmul(
                            out=ps[h * BLK : (h + 1) * BLK, :],
                            lhsT=ohs[h][:, t, :],
                            rhs=rhs.bitcast(xdt),
                            start=(t == 0),
                            stop=(t == T_b - 1),
                        )
                osl = out_sb[:, g * DIM : (g + 1) * DIM]
                nc.scalar.copy(out=osl, in_=ps[:NROW, :])
                nc.scalar.dma_start(
                    out=y[:, g * DIM : (g + 1) * DIM], in_=osl
                )
                off += NPAR * T_b
    nc.compile()
    return nc


def _pack_blocks(tcounts, T_bs, NBLK):
    """Greedy max-remaining-capacity assignment of tissues to blocks.

    Returns (tissue2block, tissue2loc) or None if some tissue does not fit
    (capacity too tight).
    """
    import heapq

    n_tissue = len(tcounts)
    nblocks_g = NCORES * NBLK
    cap = np.array(
        [P * T_bs[(b % NBLK) // NPAR] for b in range(nblocks_g)], dtype=np.int64
    )
    t_order_desc = np.argsort(-tcounts, kind="stable")
    block_sum = np.zeros(nblocks_g, dtype=np.int64)
    block_fill = np.zeros(nblocks_g, dtype=np.int64)
    tissue2block = np.empty(n_tissue, dtype=np.int64)
    tissue2loc = np.empty(n_tissue, dtype=np.int64)
    # heap of (-remaining_capacity, block)
    heap = [(-cap[b], b) for b in range(nblocks_g)]
    heapq.heapify(heap)
    for t in t_order_desc:
        c = int(tcounts[t])
        placed = False
        while heap:
            negrem, b = heapq.heappop(heap)
            if block_fill[b] >= BLK:
                continue  # out of tissue slots, drop from heap
            if -negrem < c:
                heapq.heappush(heap, (negrem, b))
                break  # max-remaining block can't fit -> infeasible
            tissue2block[t] = b
            tissue2loc[t] = block_fill[b]
            block_fill[b] += 1
            block_sum[b] += c
            heapq.heappush(heap, (-(cap[b] - block_sum[b]), b))
            placed = True
            break
        if not placed:
            return None
    return tissue2block, tissue2loc


def kernel(
    cell_features,
    tissue_features,
    cluster_assignments,
    W_cell,
    b_cell,
    W_tissue,
    b_tissue,
    attn_w,
):
    global LAST_RESULTS
    import ml_dtypes
    from concourse.bass_utils import run_bass_kernel_spmd

    cells = np.asarray(cell_features, dtype=np.float32)
    tissue = np.asarray(tissue_features, dtype=np.float32)
    assign = np.asarray(cluster_assignments).astype(np.int64)

    n_cell, DIM = cells.shape
    n_tissue = tissue.shape[0]
    TPC = -(-n_tissue // NCORES)          # tissues per core (ceil)
    NBLK = -(-TPC // BLK)                 # blocks per core
    NBLK = -(-NBLK // NPAR) * NPAR        # round to a multiple of NPAR
    NGRP = NBLK // NPAR
    nblocks_g = NCORES * NBLK

    np_xdt = ml_dtypes.float8_e3m4 if DATA_DT == "f8" else np.float16
    xq = cells.astype(np_xdt)

    # ---- host: choose per-group tile counts and pack tissues to blocks ----
    tcounts = np.bincount(assign, minlength=n_tissue)
    percore = -(-n_cell // NCORES)
    # ~1% capacity slack for the balancing greedy, largest groups first
    tiles_needed = -(-(percore + percore // 100) // (P * NPAR))
    base, extra = divmod(tiles_needed, NGRP)
    T_bs = [base + (1 if g < extra else 0) for g in range(NGRP)]
    T_bs.sort(reverse=True)
    packed = None
    while packed is None:
        packed = _pack_blocks(tcounts, T_bs, NBLK)
        if packed is None:
            T_bs[-1] += 1
            T_bs.sort(reverse=True)
    tissue2block, tissue2loc = packed

    NT = NPAR * sum(T_bs)
    NTL = NT + (NT & 1)
    Tmax = max(T_bs)

    # ---- host: sort cells by (block, position) and pack per core ----
    cell_block = tissue2block[assign]
    order = np.argsort(cell_block, kind="stable").astype(np.int64)
    sorted_block = cell_block[order]
    cuts = np.searchsorted(sorted_block, np.arange(nblocks_g + 1))
    loc_of_cell = tissue2loc[assign].astype(np.float16)

    iota_f32 = np.ascontiguousarray(
        np.tile(np.arange(BLK, dtype=np.float16), (P, Tmax))
    ).view(np.float32)

    # per-block start tile (within a core) for the variable group sizes
    tile_off = np.zeros(NBLK + 1, dtype=np.int64)
    for b in range(NBLK):
        tile_off[b + 1] = tile_off[b] + T_bs[b // NPAR]
    assert tile_off[NBLK] == NT

    in_maps = []
    for k in range(NCORES):
        pi = np.zeros(NT * P, dtype=np.int64)
        lo_ids = np.full(NTL * P, float(BLK), dtype=np.float16)  # pad -> no hit
        for b in range(NBLK):
            i = k * NBLK + b
            seg = order[cuts[i] : cuts[i + 1]]
            s0 = tile_off[b] * P
            pi[s0 : s0 + len(seg)] = seg
            lo_ids[s0 : s0 + len(seg)] = loc_of_cell[seg]
        # partition-major: x[p, t, :] = xq[pi[t*P + p]], viewed as f32 words
        x = np.ascontiguousarray(xq[pi.reshape(NT, P).T]).view(np.float32)
        locT = np.ascontiguousarray(
            lo_ids.reshape(NTL, P).T
        ).view(np.float32)
        in_maps.append({"x": x, "loc": locT, "iota": iota_f32})

    # ---- device program (cached on tiling geometry) ----
    key = (tuple(T_bs), NBLK, DIM, DATA_DT)
    nc = _PROGRAM_CACHE.get(key)
    if nc is None:
        nc = _build_program(T_bs, NBLK, DIM)
        _PROGRAM_CACHE[key] = nc

    res = run_bass_kernel_spmd(nc, in_maps, core_ids=list(range(NCORES)))
    LAST_RESULTS = res

    # ---- host: inverse-permute per-core outputs into [n_tissue, DIM] ----
    yb = np.concatenate(
        [
            res.results[k]["y"]
            .reshape(NPAR, BLK, NGRP, DIM)
            .transpose(2, 0, 1, 3)
            .reshape(NBLK, BLK, DIM)
            for k in range(NCORES)
        ],
        axis=0,
    )  # [nblocks_g, BLK, DIM] in (block, localid) layout
    out = np.ascontiguousarray(yb[tissue2block, tissue2loc]) + tissue
    return out
